# revision 2
# baseline (speedup 1.0000x reference)
"""Grouped GEMM (MoE routing) Trainium2 kernel.

Problem: x [32768, 2048] bf16, tokens pre-grouped into E=8 contiguous
segments; weights [8, 2048, 1024] bf16; splits_cpu [8] int32 segment
sizes. out[seg_e] = x[seg_e] @ weights[e], fp32 accumulation, bf16 out.

Strategy: expert-parallel over 8 NeuronCores. Core e gets its expert's
token segment (host-sliced, host-transposed to K-major tiles) plus
weights[e], and runs a dense 4096x2048x1024 matmul:
  - dual-ring startup: x tiles stream on the sync HWDGE ring, w chunks
    on the scalar HWDGE ring, so both make progress concurrently; the
    first-block x tiles and the first two w chunks are split fine
    (x0 in ko-quarters, x1/x2 in halves; w0/w1 in n-halves) so the
    first real matmul's operands (x0[ko 0:4] + w0[:, 0:512]) land
    ~8.9 us instead of ~11.6 us.
  - HAM warmup: dummy matmuls fed by a DVE memset keep the PE busy
    from ~7.5 us so the 2.4 GHz clock-gate opens right as real data
    lands; the real matmul chase continues the busy window.
  - first block: 4 m-tiles x 16 ko on 8 PSUM banks, matmuls emitted in
    modeled chunk-arrival order so the PE chases the two incoming
    streams.
  - steady state: per m-tile, 2 PSUM banks (N=512 each), 16-step K
    accumulation, PSUM -> bf16 SBUF copy on ACT/DVE, DMA out on sync.
  - tail: last m-tile runs n-major; cols 512:1024 accumulate in TWO
    256-wide PSUM banks whose evicts run concurrently on scalar and
    vector the moment each bank's last matmul retires, with output
    DMAs on the sync and scalar rings.
Compute bound: 1024 matmuls of 128x128x512 per core; steady spacing is
N/2.4GHz + ~2.5ns NX overhead = ~216 ns -> ~221 us stream.
"""

import numpy as np

P = 128
E = 8
K_DIM = 2048
N_DIM = 1024
KO_TILES = K_DIM // P  # 16
WARMUP_MMS = 5

_CACHE = {}


def _startup_schedule(block, ko_tiles):
    """Model the two HWDGE rings and return (x_chunks, w_chunks, mm_order).

    x_chunks: list of (mo, ko_lo, ko_hi) DMA pieces on the sync ring.
    w_chunks: list of (ko, n_lo, n_hi) DMA pieces on the scalar ring.
    mm_order: list of (mo, ko, half) triples sorted by modeled arrival
    of max(x piece, w piece); within each (mo, half) the ko order stays
    monotonic, so start/stop accumulate flags land on ko==0/ko==last.
    """
    x_chunks = []
    if block >= 1:
        x_chunks += [(0, 0, 4), (0, 4, 8), (0, 8, 12), (0, 12, 16)]
    if block >= 2:
        x_chunks += [(1, 0, 8), (1, 8, 16)]
    if block >= 3:
        x_chunks += [(2, 0, 8), (2, 8, 16)]
    for mo in range(3, block):
        x_chunks.append((mo, 0, 16))

    w_chunks = []
    for ko in range(ko_tiles):
        if ko < 2:
            w_chunks += [(ko, 0, 512), (ko, 512, 1024)]
        else:
            w_chunks.append((ko, 0, 1024))

    # arrival model: both rings share HBM ~evenly -> ~0.18 GB/us each;
    # x tile ko-span of 4 = 128 KiB = ~0.72 us; w 512 cols = ~0.72 us.
    t = 0.0
    t_x = {}
    for mo, lo, hi in x_chunks:
        t += 0.18 * (hi - lo)
        for ko in range(lo, hi):
            t_x[(mo, ko)] = t
    t = 0.0
    t_w = {}
    for ko, nlo, nhi in w_chunks:
        t += 0.0028 * (nhi - nlo)
        for h in range(2):
            if nlo <= h * 512 < nhi:
                t_w[(ko, h)] = t

    pairs = [
        (mo, ko, h)
        for mo in range(block)
        for ko in range(ko_tiles)
        for h in range(2)
    ]
    pairs.sort(key=lambda p: (max(t_x[(p[0], p[1])], t_w[(p[1], p[2])]),
                              p[1], p[0], p[2]))
    return x_chunks, w_chunks, pairs


def _build(mo_tiles):
    """Build + bacc-compile the per-core Bass program for mo_tiles m-tiles."""
    import concourse.mybir as mybir
    import concourse.tile as tile
    from concourse import bacc

    nc = bacc.Bacc("TRN2", target_bir_lowering=False, debug=False)
    dt = mybir.dt.bfloat16
    f32 = mybir.dt.float32

    # xt[mo, p, ko, mi] = x_seg[mo*128 + mi, ko*128 + p]
    xt = nc.dram_tensor("xt", [mo_tiles, P, KO_TILES, P], dt, kind="ExternalInput").ap()
    # w[p, ko, n] = w_e[ko*128 + p, n]
    w = nc.dram_tensor("w", [P, KO_TILES, N_DIM], dt, kind="ExternalInput").ap()
    # out[mo, p, n] = out_seg[mo*128 + p, n]
    out = nc.dram_tensor("out", [mo_tiles, P, N_DIM], dt, kind="ExternalOutput").ap()

    BLOCK = min(4, mo_tiles)
    x_chunks, w_chunks, mm_order = _startup_schedule(BLOCK, KO_TILES)

    with tile.TileContext(nc) as tc:
        with (
            tc.tile_pool(name="const", bufs=1) as cpool,
            tc.tile_pool(name="wpool", bufs=1) as wpool,
            tc.tile_pool(name="xpool", bufs=10) as xpool,
            tc.tile_pool(name="opool", bufs=4) as opool,
            tc.tile_pool(name="psum", bufs=8, space="PSUM") as pspool,
        ):
            # --- startup DMAs first in program order so both rings begin
            # issuing right at the tile-context entry barrier.
            xq = [xpool.tile([P, KO_TILES, P], dt, tag="x", name=f"x_{mo}")
                  for mo in range(BLOCK)]
            w_sb = wpool.tile([P, KO_TILES, N_DIM], dt)
            for mo, lo, hi in x_chunks:
                nc.sync.dma_start(xq[mo][:, lo:hi, :], xt[mo][:, lo:hi, :])
            for ko, nlo, nhi in w_chunks:
                nc.scalar.dma_start(w_sb[:, ko, nlo:nhi], w[:, ko, nlo:nhi])

            # --- HAM warmup: dummy matmuls fed by a DVE memset keep the
            # PE busy early so the 2.4 GHz clock-gate opens as real data
            # lands.
            dummy = cpool.tile([P, 640], dt)
            nc.vector.memset(dummy[:], 0.0)
            warm_ps = pspool.tile([P, 512], f32, tag="ps")
            for _ in range(WARMUP_MMS):
                nc.tensor.matmul(warm_ps[:], dummy[:, 0:P], dummy[:, P:640],
                                 start=True, stop=True)

            def issue_x(mo):
                t = xpool.tile([P, KO_TILES, P], dt, tag="x", name=f"x_{mo}")
                nc.sync.dma_start(t[:], xt[mo])
                xq.append(t)

            def evict(ps0, ps1, mo):
                o_sb = opool.tile([P, N_DIM], dt, tag="o")
                nc.scalar.copy(o_sb[:, 0:512], ps0[:])
                nc.vector.tensor_copy(o_sb[:, 512:1024], ps1[:])
                nc.sync.dma_start(out[mo], o_sb[:])

            # --- first block: matmuls in modeled chunk-arrival order
            # across BLOCK m-tiles / 2*BLOCK PSUM banks.
            pss = [
                [
                    pspool.tile([P, 512], f32, tag="ps", name=f"ps_{mo}_{h}")
                    for h in range(2)
                ]
                for mo in range(BLOCK)
            ]
            seen = {}
            for mo, ko, h in mm_order:
                key = (mo, h)
                cnt = seen.get(key, 0)
                seen[key] = cnt + 1
                nc.tensor.matmul(pss[mo][h][:], xq[mo][:, ko, :],
                                 w_sb[:, ko, 512 * h:512 * (h + 1)],
                                 start=cnt == 0, stop=cnt == KO_TILES - 1)
            for mo in range(BLOCK):
                evict(pss[mo][0], pss[mo][1], mo)

            # steady-state prefetches in program order; pool slots gate depth
            for mo in range(BLOCK, mo_tiles):
                issue_x(mo)

            # --- steady state: per m-tile, mo-major
            last_mo = mo_tiles - 1
            for mo in range(BLOCK, last_mo):
                x_sb = xq[mo]
                ps0 = pspool.tile([P, 512], f32, tag="ps")
                ps1 = pspool.tile([P, 512], f32, tag="ps")
                for ko in range(KO_TILES):
                    first = ko == 0
                    last = ko == KO_TILES - 1
                    lhsT = x_sb[:, ko, :]
                    nc.tensor.matmul(ps0[:], lhsT, w_sb[:, ko, 0:512],
                                     start=first, stop=last)
                    nc.tensor.matmul(ps1[:], lhsT, w_sb[:, ko, 512:1024],
                                     start=first, stop=last)
                evict(ps0, ps1, mo)

            # --- last m-tile: n-major; cols 0:512 evict+DMA overlap the
            # second half, which runs in two 256-wide PSUM banks so the
            # final evicts parallelize on scalar+vector and the two
            # output DMAs ride different rings.
            if last_mo >= BLOCK:
                x_sb = xq[last_mo]
                ps0 = pspool.tile([P, 512], f32, tag="ps")
                psa = pspool.tile([P, 256], f32, tag="ps")
                psb = pspool.tile([P, 256], f32, tag="ps")
                o_sb = opool.tile([P, N_DIM], dt, tag="o")
                for ko in range(KO_TILES):
                    nc.tensor.matmul(ps0[:], x_sb[:, ko, :], w_sb[:, ko, 0:512],
                                     start=ko == 0, stop=ko == KO_TILES - 1)
                nc.scalar.copy(o_sb[:, 0:512], ps0[:])
                nc.scalar.dma_start(out[last_mo][:, 0:512], o_sb[:, 0:512])
                for ko in range(KO_TILES):
                    nc.tensor.matmul(psa[:], x_sb[:, ko, :], w_sb[:, ko, 512:768],
                                     start=ko == 0, stop=ko == KO_TILES - 1)
                    nc.tensor.matmul(psb[:], x_sb[:, ko, :], w_sb[:, ko, 768:1024],
                                     start=ko == 0, stop=ko == KO_TILES - 1)
                nc.scalar.copy(o_sb[:, 512:768], psa[:])
                nc.vector.tensor_copy(o_sb[:, 768:1024], psb[:])
                nc.sync.dma_start(out[last_mo][:, 512:768], o_sb[:, 512:768])
                nc.scalar.dma_start(out[last_mo][:, 768:1024], o_sb[:, 768:1024])

    nc.compile()
    return nc


def _get_nc(mo_tiles):
    if mo_tiles not in _CACHE:
        _CACHE[mo_tiles] = _build(mo_tiles)
    return _CACHE[mo_tiles]


def run(input, weights, splits_cpu, trace=False):
    import ml_dtypes
    from concourse.bass_utils import run_bass_kernel_spmd

    x = np.asarray(input)
    wts = np.asarray(weights)
    splits = [int(s) for s in np.asarray(splits_cpu)]
    assert len(splits) == E and sum(splits) == x.shape[0]
    bf16 = ml_dtypes.bfloat16

    seg_cap = max(max(splits), P)
    seg_cap = -(-seg_cap // P) * P  # round up to multiple of 128
    mo_tiles = seg_cap // P

    starts = np.cumsum([0] + splits)
    in_maps = []
    for e in range(E):
        xe = x[starts[e]:starts[e + 1]]
        if xe.shape[0] < seg_cap:
            pad = np.zeros((seg_cap - xe.shape[0], K_DIM), dtype=bf16)
            xe = np.concatenate([xe.astype(bf16), pad], axis=0)
        # [S, K] -> [mo, p, ko, mi]
        xt = np.ascontiguousarray(
            xe.astype(bf16).reshape(mo_tiles, P, KO_TILES, P).transpose(0, 3, 2, 1)
        )
        we = np.ascontiguousarray(
            wts[e].astype(bf16).reshape(KO_TILES, P, N_DIM).transpose(1, 0, 2)
        )
        in_maps.append({"xt": xt, "w": we})

    nc = _get_nc(mo_tiles)
    res = run_bass_kernel_spmd(nc, in_maps, core_ids=list(range(E)), trace=trace)

    outs = []
    for e in range(E):
        oe = np.asarray(res.results[e]["out"]).reshape(seg_cap, N_DIM)
        outs.append(oe[: splits[e]])
    full = np.concatenate(outs, axis=0).astype(x.dtype)
    return full, res.exec_time_ns


def kernel(input, weights, splits_cpu):
    out, _ = run(input, weights, splits_cpu, trace=False)
    return out
